# revision 28
# baseline (speedup 1.0000x reference)
"""Trainium2 Bass kernel for block-local (sparse window) attention.

Problem: B=4, S=4096, DIM=768, H=12 heads x DH=64, local window W=256.
    out = (softmax_blocklocal(mask(Q K^T / sqrt(DH))) V) @ Wff + bff

Sharding: 8 cores, core c = (batch c//2, sequence half c%2) -> 2048 tokens
per core = 8 complete 256-token blocks. Projections are per-token, attention
is block-local, FF is per-token => embarrassingly parallel, no collectives.

Per-core kernel (all feature-major to avoid transposes; bf16 matmuls):
  X^T [768,2048] (host-pretransposed, host-pretiled bf16)
  Q^T/K^T = lhsT=Wq/Wk [dim,hd] (natural layout), rhs=X^T -> [hd,t]; bias via
    DVE per-partition tensor_scalar add on the PSUM->SBUF copy.
  V = token-major [t,hd]: lhsT=X^T chunk, rhs=Wv; key-padding mask folded in
    via per-partition multiply on the copy (V rows of masked keys zeroed).
  Attention per block, per head pair hp:
    A: 4 score matmuls scores^T[k,q] (K=64 row tiling, tile_position
      auto-derived (0,0)/(64,0) so the T0/T8 pair streams concurrently)
      into ONE 2-bank psum tile [128, par, kc, 256] (parity = bank).
    exp: ONE ACT op per head pair over all 1024 cols, scale=1/8 -> bf16 et.
      The single consumer keeps the 4 matmuls dense in the PE stream so
      the row-tile pairs actually pack (measured pair ~= one matmul time).
    B (128x64 col tiling, (0,0)/(0,64) pairs concurrent): av (cols 0:256,
      parities in partition halves, kc-outer/par-inner so pairs pack) and
      dp = lhsT=mask-replicated [k,64] @ E^T -> denominator REPLICATED
      across the 64 partitions of its head's half (PE does the broadcast).
    One DVE reciprocal + one DVE multiply per pair normalizes both heads.
  out^T[o,t] = lhsT=Wff[hd,o] (natural), rhs=attn^T; bias=bff+bv@Wff (host-
  folded, exact because softmax rows sum to 1) on the ACT copy.
  Emission is software-pipelined in 4 token-quarter units:
    proj(u) -> attn(block 2u, 2u+1) -> FF(u)
  so ACT exp work overlaps PE projection/FF matmuls of neighboring units
  (the Tile scheduler interleaves the per-engine streams).

  I/O: the DMA rings process ~one descriptor per (partition, contiguous
  segment); with row-major DRAM layouts every 128-partition transfer cost
  128 small descriptors and the load/store path was descriptor-bound, not
  HBM-bound.  All big tensors are therefore host-pretiled so each SBUF
  partition's data is contiguous in DRAM (one fat descriptor per
  partition), transfers are split (dc-pair x partition-half) across the
  SP and ACT HWDGE queues for ring parallelism, and ordered by first use.
  26 dummy matmuls on a memset tile run during the load so the PE HAM
  clock gate is already 8/8 (2.4 GHz) when the first real matmul issues.
  Output stores are oc-pair [128,1024] bf16 tiles, partition-contiguous
  in DRAM, split by partition halves (quarters for the last unit) so the
  final drain is short.  Host reassembles [tt,p,oc,512] -> [t,d].
"""

import numpy as np
import ml_dtypes

import concourse.bass as bass
import concourse.mybir as mybir
from concourse import bacc
from concourse.tile import TileContext
from concourse.bass_utils import run_bass_kernel_spmd

B, S, DIM = 4, 4096, 768
H, DH = 12, 64
W = 256
NCORES = 8
T = (B * S) // NCORES       # 2048 tokens per core
NB = T // W                 # 8 blocks per core
NKC = T // 128              # 16 token chunks of 128 per core
DC = DIM // 128             # 6 dim chunks
HC = (H * DH) // 128        # 6 hd chunks
BF16 = mybir.dt.bfloat16
F32 = mybir.dt.float32

_nc_cache = {}


def _build_nc():
    nc = bacc.Bacc()

    # All big tensors are host-pre-tiled so that each SBUF partition's data
    # is CONTIGUOUS in DRAM: one fat DMA descriptor per partition instead of
    # one 0.5-1.5KB descriptor per (partition, chunk).  The DMA rings process
    # descriptors at ~20M/s, so descriptor count - not bytes - dominated the
    # load/store time in the row-major layout.
    # xt: [tt, p, dc*512] (token quarter major), weights: [p, dc*DIM].
    xt_d = nc.declare_dram_parameter("xt", [4, 128, DC * 512], BF16, isOutput=False)
    wq_d = nc.declare_dram_parameter("wq", [128, DC * DIM], BF16, isOutput=False)
    wk_d = nc.declare_dram_parameter("wk", [128, DC * DIM], BF16, isOutput=False)
    wv_d = nc.declare_dram_parameter("wv", [128, DC * DIM], BF16, isOutput=False)
    wff_d = nc.declare_dram_parameter("wff", [128, DC * DIM], BF16, isOutput=False)
    bq_d = nc.declare_dram_parameter("bq", [128, HC], F32, isOutput=False)
    bk_d = nc.declare_dram_parameter("bk", [128, HC], F32, isOutput=False)
    bffe_d = nc.declare_dram_parameter("bffe", [128, DC], F32, isOutput=False)
    # mask as 0/1: per-partition scalar [128, NKC] and 64-wide replicated bf16
    mv_d = nc.declare_dram_parameter("mv", [128, NKC], F32, isOutput=False)
    mbc_d = nc.declare_dram_parameter("mbc", [128, NKC * 64], BF16, isOutput=False)
    # out: [tt, p, oc*512] (token quarter major, partition-contiguous)
    out_d = nc.declare_dram_parameter("out", [4, 128, DC * 512], BF16, isOutput=True)

    Exp = mybir.ActivationFunctionType.Exp
    Ident = mybir.ActivationFunctionType.Identity
    ADD = mybir.AluOpType.add
    MULT = mybir.AluOpType.mult

    with TileContext(nc) as tc:
        with (
            tc.tile_pool(name="const", bufs=1) as cpool,
            tc.tile_pool(name="mm", bufs=2, space="PSUM") as mm_pool,
            tc.tile_pool(name="sps", bufs=2, space="PSUM") as s_pool,
            tc.tile_pool(name="adp", bufs=2, space="PSUM") as ad_pool,
            tc.tile_pool(name="et", bufs=7) as et_pool,
            tc.tile_pool(name="nrm", bufs=4) as nrm_pool,
            tc.tile_pool(name="ob", bufs=3) as ob_pool,
        ):
            # ---- persistent SBUF tensors ----
            # xt: token-quarter major [128, tt, dc, 512] to match the
            # pre-tiled DRAM layout (one fat descriptor per partition)
            xt_sb = cpool.tile([128, 4, DC, 512], BF16, name="xt_sb")
            wq_sb = cpool.tile([128, DC, DIM], BF16, name="wq_sb")
            wk_sb = cpool.tile([128, DC, DIM], BF16, name="wk_sb")
            wv_sb = cpool.tile([128, DC, DIM], BF16, name="wv_sb")
            wff_sb = cpool.tile([128, HC, DIM], BF16, name="wff_sb")
            qt_sb = cpool.tile([128, HC, T], BF16, name="qt_sb")
            kt_sb = cpool.tile([128, HC, T], BF16, name="kt_sb")
            v_sb = cpool.tile([128, NKC, DIM], BF16, name="v_sb")
            at_sb = cpool.tile([128, HC, T], BF16, name="at_sb")
            bq_sb = cpool.tile([128, HC], F32, name="bq_sb")
            bk_sb = cpool.tile([128, HC], F32, name="bk_sb")
            bffe_sb = cpool.tile([128, DC], F32, name="bffe_sb")
            mv_sb = cpool.tile([128, NKC], F32, name="mv_sb")
            mbc_sb = cpool.tile([128, NKC, 64], BF16, name="mbc_sb")

            # ---- PE warm-up: dummy matmuls on a memset tile run while the
            # input DMAs stream, so the HAM clock gate is already at 8/8
            # (2.4 GHz) when the first real matmul issues, and the PE is not
            # idle during the load.
            warm_sb = cpool.tile([128, 128], BF16, name="warm_sb")
            nc.vector.memset(warm_sb[:], 0.0)
            wps = mm_pool.tile([128, 512], F32, tag="mm", name="wps")
            for _ in range(26):
                nc.tensor.matmul(
                    wps[:, 0:128], warm_sb[:], warm_sb[:], start=True, stop=True
                )

            # ---- load inputs ----
            xt_v = xt_d.ap().rearrange("t p (c q) -> t p c q", q=512)
            wq_v = wq_d.ap().rearrange("p (c o) -> p c o", o=DIM)
            wk_v = wk_d.ap().rearrange("p (c o) -> p c o", o=DIM)
            wv_v = wv_d.ap().rearrange("p (c o) -> p c o", o=DIM)
            wff_v = wff_d.ap().rearrange("p (c o) -> p c o", o=DIM)
            # The first Q-projection needs wq + xt quarter 0.  Each transfer
            # covering all 128 partitions costs >=128 descriptors of ring
            # latency, so the critical first tensors are split into
            # (dc-pair x partition-half) pieces across both HWDGE queues so
            # several rings work in parallel.
            qeng = [nc.sync, nc.scalar]
            # wq is loaded column-half first: the first three Q psum groups
            # (hc 0-2) only need cols 0:384 of every dc chunk, so Q starts
            # after half the wq bytes have landed.
            for half in range(2):
                c0, c1 = half * 384, half * 384 + 384
                for i in range(3):
                    for h in range(2):
                        p0, p1 = h * 64, (h + 1) * 64
                        qeng[(h + half) % 2].dma_start(
                            out=wq_sb[p0:p1, 2 * i:2 * i + 2, c0:c1],
                            in_=wq_v[p0:p1, 2 * i:2 * i + 2, c0:c1],
                        )
                        if half == 0:
                            qeng[1 - h].dma_start(
                                out=xt_sb[p0:p1, 0, 2 * i:2 * i + 2, :],
                                in_=xt_v[0, p0:p1, 2 * i:2 * i + 2, :],
                            )
            nc.scalar.dma_start(out=bq_sb[:], in_=bq_d.ap())
            nc.scalar.dma_start(out=bk_sb[:], in_=bk_d.ap())
            nc.scalar.dma_start(out=mv_sb[:], in_=mv_d.ap())
            for half in range(2):
                c0, c1 = half * 384, half * 384 + 384
                for i in range(3):
                    for h in range(2):
                        p0, p1 = h * 64, (h + 1) * 64
                        qeng[(i + h + half) % 2].dma_start(
                            out=wk_sb[p0:p1, 2 * i:2 * i + 2, c0:c1],
                            in_=wk_v[p0:p1, 2 * i:2 * i + 2, c0:c1],
                        )
            for h in range(2):
                p0, p1 = h * 64, (h + 1) * 64
                qeng[h].dma_start(out=wv_sb[p0:p1], in_=wv_v[p0:p1])
            for tt in range(1, 4):
                for h in range(2):
                    p0, p1 = h * 64, (h + 1) * 64
                    qeng[(tt + h) % 2].dma_start(
                        out=xt_sb[p0:p1, tt], in_=xt_v[tt, p0:p1]
                    )
            nc.scalar.dma_start(
                out=mbc_sb[:], in_=mbc_d.ap().rearrange("p (c o) -> p c o", o=64)
            )
            nc.scalar.dma_start(out=bffe_sb[:], in_=bffe_d.ap())
            for h in range(2):
                p0, p1 = h * 64, (h + 1) * 64
                qeng[h].dma_start(out=wff_sb[p0:p1], in_=wff_v[p0:p1])

            def proj_qk(w_sb, b_sb, o_sb, tt):
                # one token-quarter of a Q^T/K^T projection: out [hd, 512]
                for hc in range(HC):
                    ps = mm_pool.tile([128, 512], F32, tag="mm", name="ps")
                    for dc in range(DC):
                        nc.tensor.matmul(
                            ps[:],
                            w_sb[:, dc, hc * 128:(hc + 1) * 128],
                            xt_sb[:, tt, dc],
                            start=(dc == 0),
                            stop=(dc == DC - 1),
                        )
                    nc.vector.tensor_scalar(
                        out=o_sb[:, hc, tt * 512:(tt + 1) * 512],
                        in0=ps[:],
                        scalar1=b_sb[:, hc:hc + 1],
                        scalar2=None,
                        op0=ADD,
                    )

            def proj_v(kc):
                # V token-chunk [128 tokens, 768], mask folded in.
                # dc outer / half inner so consecutive matmuls share lhsT.
                tt, j0 = kc // 4, (kc % 4) * 128
                ps = [
                    mm_pool.tile([128, 384], F32, tag="mm", name="ps"),
                    mm_pool.tile([128, 384], F32, tag="mm", name="ps"),
                ]
                for dc in range(DC):
                    for half in range(2):
                        nc.tensor.matmul(
                            ps[half][:],
                            xt_sb[:, tt, dc, j0:j0 + 128],
                            wv_sb[:, dc, half * 384:(half + 1) * 384],
                            start=(dc == 0),
                            stop=(dc == DC - 1),
                        )
                for half in range(2):
                    nc.vector.tensor_scalar(
                        out=v_sb[:, kc, half * 384:(half + 1) * 384],
                        in0=ps[half][:],
                        scalar1=mv_sb[:, kc:kc + 1],
                        scalar2=None,
                        op0=MULT,
                    )

            def attn_block(blk):
                q0 = blk * 256

                # --- phase A: all scores (row-tiled 64x128, T0/T8 pack) ---
                # sp is a [128, par, kc, 256] 2-bank tile per head pair:
                # parity selects the bank (row tiles must hit different
                # banks), kc the half within it.  One EXP per head pair
                # covers all 1024 columns: fewer ACT ops, and the single
                # consumer keeps the 4 matmuls dense in the PE stream so
                # the T0/T8 pairs actually pack.
                ets = []
                for hp in range(H // 2):
                    sp = s_pool.tile([128, 2, 2, 256], F32, tag="s", name="sp")
                    for kc in range(2):
                        k0 = q0 + kc * 128
                        for par in range(2):  # alternate T0/T8 for packing
                            hr = par * 64
                            nc.tensor.matmul(
                                sp[:, par, kc],
                                kt_sb[hr:hr + 64, hp, k0:k0 + 128],
                                qt_sb[hr:hr + 64, hp, q0:q0 + 256],
                                start=True, stop=True,
                            )
                    et = et_pool.tile([128, 2, 2, 256], BF16, tag="et", name="et")
                    nc.scalar.activation(
                        et[:], sp[:], Exp, bias=0.0, scale=0.125
                    )
                    ets.append(et)
                # --- phase B: av (cols 0:256) + denominator (cols 256:512)
                # in one bank, col-tiled 128x64 T0/T1; parities alternate
                # innermost so the two col tiles stream concurrently ---
                for hp in range(H // 2):
                    ad = ad_pool.tile([128, 512], F32, tag="ad", name="ad")
                    for kc in range(2):
                        tkc = blk * 2 + kc
                        for par in range(2):
                            hr = par * 64
                            h = 2 * hp + par
                            nc.tensor.matmul(
                                ad[hr:hr + 64, 0:256],
                                v_sb[:, tkc, h * 64:(h + 1) * 64],
                                ets[hp][:, par, kc],
                                start=(kc == 0), stop=(kc == 1),
                                skip_group_check=True,
                            )
                    for kc in range(2):
                        tkc = blk * 2 + kc
                        for par in range(2):
                            hr = par * 64
                            nc.tensor.matmul(
                                ad[hr:hr + 64, 256:512],
                                mbc_sb[:, tkc],
                                ets[hp][:, par, kc],
                                start=(kc == 0), stop=(kc == 1),
                                skip_group_check=True,
                            )
                    rc = nrm_pool.tile([128, 256], F32, tag="rc", name="rc")
                    nc.vector.reciprocal_approx_fast(rc[:], ad[:, 256:512])
                    nc.vector.tensor_mul(
                        at_sb[:, hp, q0:q0 + 256], ad[:, 0:256], rc[:]
                    )

            out_v = out_d.ap().rearrange("t p (j q) -> t p j q", q=1024)

            def ff(tt):
                # oc pairs share one [128, 1024] bf16 tile; the store for a
                # pair is partition-contiguous 2KB in DRAM, split by
                # partition halves (quarters for the last unit) so several
                # rings drain it in parallel.
                ob = None
                for oc in range(DC):
                    ps = mm_pool.tile([128, 512], F32, tag="mm", name="ps")
                    for hc in range(HC):
                        nc.tensor.matmul(
                            ps[:],
                            wff_sb[:, hc, oc * 128:(oc + 1) * 128],
                            at_sb[:, hc, tt * 512:(tt + 1) * 512],
                            start=(hc == 0),
                            stop=(hc == HC - 1),
                        )
                    if oc % 2 == 0:
                        ob = ob_pool.tile([128, 1024], BF16, tag="ob", name="ob")
                    if tt == 3 and oc % 2 == 1:
                        # last unit: odd-oc copies go to the DVE so the final
                        # PSUM evacuations run on two engines in parallel
                        nc.vector.tensor_scalar(
                            out=ob[:, 512:1024], in0=ps[:],
                            scalar1=bffe_sb[:, oc:oc + 1], scalar2=None,
                            op0=ADD,
                        )
                    else:
                        nc.scalar.activation(
                            ob[:, (oc % 2) * 512:(oc % 2) * 512 + 512], ps[:],
                            Ident, bias=bffe_sb[:, oc:oc + 1], scale=1.0
                        )
                    if oc % 2 == 1:
                        j = oc // 2
                        nsp = 8 if tt == 3 else 2
                        step = 128 // nsp
                        for h in range(nsp):
                            p0, p1 = h * step, (h + 1) * step
                            qeng[(j + h) % 2].dma_start(
                                out=out_v[tt, p0:p1, j],
                                in_=ob[p0:p1, :],
                            )

            # ---- software-pipelined emission over 4 token-quarters ----
            for u in range(4):
                proj_qk(wq_sb, bq_sb, qt_sb, u)
                proj_qk(wk_sb, bk_sb, kt_sb, u)
                for kc in range(4 * u, 4 * u + 4):
                    proj_v(kc)
                attn_block(2 * u)
                attn_block(2 * u + 1)
                ff(u)

    nc.finalize()
    return nc


def _get_nc():
    if "nc" not in _nc_cache:
        _nc_cache["nc"] = _build_nc()
    return _nc_cache["nc"]


def _tile_w(w):
    # [DIM, DIM] row-major -> [128, DC*DIM] with each partition's DC chunks
    # contiguous: element (dc*128+p, o) -> arr[p, dc*DIM+o]
    bf = ml_dtypes.bfloat16
    return np.ascontiguousarray(
        w.astype(bf).reshape(DC, 128, DIM).transpose(1, 0, 2).reshape(128, DC * DIM)
    )


def _prep_in_maps(X, mask, Wq, bq, Wk, bk, Wv, bv, Wff, bff):
    bf = ml_dtypes.bfloat16
    wq_b = _tile_w(Wq)
    wk_b = _tile_w(Wk)
    wv_b = _tile_w(Wv)
    wff_b = _tile_w(Wff)
    # per-partition bias layouts: [128, nchunks] with col = chunk
    bq_t = np.ascontiguousarray(bq.astype(np.float32).reshape(HC, 128).T)
    bk_t = np.ascontiguousarray(bk.astype(np.float32).reshape(HC, 128).T)
    bffe = (bff.astype(np.float64)
            + bv.astype(np.float64) @ Wff.astype(np.float64)).astype(np.float32)
    bffe_t = np.ascontiguousarray(bffe.reshape(DC, 128).T)

    in_maps = []
    for c in range(NCORES):
        b, s0 = divmod(c, 2)
        s0 *= T
        xt = X[b, s0:s0 + T, :].T.astype(bf)   # [DIM, T]
        # -> [tt, p, dc*512]: element (dc*128+p, tt*512+q) -> arr[tt, p, dc, q]
        xt_t = np.ascontiguousarray(
            xt.reshape(DC, 128, 4, 512).transpose(2, 1, 0, 3).reshape(4, 128, DC * 512)
        )
        mvalid = (mask[b, s0:s0 + T] > 0).astype(np.float32)  # [T] 0/1
        mv_t = np.ascontiguousarray(mvalid.reshape(NKC, 128).T)  # [128, NKC]
        mbc = np.ascontiguousarray(
            np.broadcast_to(mv_t[:, :, None], (128, NKC, 64))
            .reshape(128, NKC * 64).astype(bf))
        in_maps.append({
            "xt": xt_t, "wq": wq_b, "wk": wk_b, "wv": wv_b, "wff": wff_b,
            "bq": bq_t, "bk": bk_t, "bffe": bffe_t,
            "mv": mv_t, "mbc": mbc,
        })
    return in_maps


def _assemble(results):
    out = np.empty((B, S, DIM), np.float32)
    for c in range(NCORES):
        b, s0 = divmod(c, 2)
        s0 *= T
        # [tt, p, oc*512] -> [t, d]: out[tt*512+q, oc*128+p] = arr[tt, p, oc, q]
        a = results[c]["out"].reshape(4, 128, DC, 512)
        out[b, s0:s0 + T, :] = (
            a.transpose(0, 3, 2, 1).reshape(T, DIM).astype(np.float32)
        )
    return out


def run(trace=False, **inputs):
    nc = _get_nc()
    in_maps = _prep_in_maps(**inputs)
    res = run_bass_kernel_spmd(
        nc, in_maps, core_ids=list(range(NCORES)), trace=trace
    )
    return _assemble(res.results), res


def kernel(**inputs) -> np.ndarray:
    out, _ = run(trace=False, **inputs)
    return out



# revision 30
# speedup vs baseline: 1.0157x; 1.0157x over previous
"""Trainium2 Bass kernel for block-local (sparse window) attention.

Problem: B=4, S=4096, DIM=768, H=12 heads x DH=64, local window W=256.
    out = (softmax_blocklocal(mask(Q K^T / sqrt(DH))) V) @ Wff + bff

Sharding: 8 cores, core c = (batch c//2, sequence half c%2) -> 2048 tokens
per core = 8 complete 256-token blocks. Projections are per-token, attention
is block-local, FF is per-token => embarrassingly parallel, no collectives.

Per-core kernel (all feature-major to avoid transposes; bf16 matmuls):
  X^T [768,2048] (host-pretransposed, host-pretiled bf16)
  Q^T/K^T = lhsT=Wq/Wk [dim,hd] (natural layout), rhs=X^T -> [hd,t]; bias via
    DVE per-partition tensor_scalar add on the PSUM->SBUF copy.
  V = token-major [t,hd]: lhsT=X^T chunk, rhs=Wv; key-padding mask folded in
    via per-partition multiply on the copy (V rows of masked keys zeroed).
  Attention per block, per head pair hp:
    A: 4 score matmuls scores^T[k,q] (K=64 row tiling, tile_position
      auto-derived (0,0)/(64,0) so the T0/T8 pair streams concurrently)
      into ONE 2-bank psum tile [128, par, kc, 256] (parity = bank).
    exp: ONE ACT op per head pair over all 1024 cols, scale=1/8 -> bf16 et.
      The single consumer keeps the 4 matmuls dense in the PE stream so
      the row-tile pairs actually pack (measured pair ~= one matmul time).
    B (128x64 col tiling, (0,0)/(0,64) pairs concurrent): av (cols 0:256,
      parities in partition halves, kc-outer/par-inner so pairs pack) and
      dp = lhsT=mask-replicated [k,64] @ E^T -> denominator REPLICATED
      across the 64 partitions of its head's half (PE does the broadcast).
    One DVE reciprocal + one DVE multiply per pair normalizes both heads.
  out^T[o,t] = lhsT=Wff[hd,o] (natural), rhs=attn^T; bias=bff+bv@Wff (host-
  folded, exact because softmax rows sum to 1) on the ACT copy.
  Emission is software-pipelined in 4 token-quarter units:
    proj(u) -> attn(block 2u, 2u+1) -> FF(u)
  so ACT exp work overlaps PE projection/FF matmuls of neighboring units
  (the Tile scheduler interleaves the per-engine streams).

  I/O: the DMA rings process ~one descriptor per (partition, contiguous
  segment); with row-major DRAM layouts every 128-partition transfer cost
  128 small descriptors and the load/store path was descriptor-bound, not
  HBM-bound.  All big tensors are therefore host-pretiled so each SBUF
  partition's data is contiguous in DRAM (one fat descriptor per
  partition), transfers are split (dc-pair x partition-half) across the
  SP and ACT HWDGE queues for ring parallelism, and ordered by first use.
  26 dummy matmuls on a memset tile run during the load so the PE HAM
  clock gate is already 8/8 (2.4 GHz) when the first real matmul issues.
  Output stores are oc-pair [128,1024] bf16 tiles, partition-contiguous
  in DRAM, split by partition halves (quarters for the last unit) so the
  final drain is short.  Host reassembles [tt,p,oc,512] -> [t,d].
"""

import numpy as np
import ml_dtypes

import concourse.bass as bass
import concourse.mybir as mybir
from concourse import bacc
from concourse.tile import TileContext
from concourse.bass_utils import run_bass_kernel_spmd

B, S, DIM = 4, 4096, 768
H, DH = 12, 64
W = 256
NCORES = 8
T = (B * S) // NCORES       # 2048 tokens per core
NB = T // W                 # 8 blocks per core
NKC = T // 128              # 16 token chunks of 128 per core
DC = DIM // 128             # 6 dim chunks
HC = (H * DH) // 128        # 6 hd chunks
BF16 = mybir.dt.bfloat16
F32 = mybir.dt.float32

_nc_cache = {}


def _build_nc():
    nc = bacc.Bacc()

    # All big tensors are host-pre-tiled so that each SBUF partition's data
    # is CONTIGUOUS in DRAM: one fat DMA descriptor per partition instead of
    # one 0.5-1.5KB descriptor per (partition, chunk).  The DMA rings process
    # descriptors at ~20M/s, so descriptor count - not bytes - dominated the
    # load/store time in the row-major layout.
    # xt: [tt, p, dc*512] (token quarter major), weights: [p, dc*DIM].
    xt_d = nc.declare_dram_parameter("xt", [4, 128, DC * 512], BF16, isOutput=False)
    wq_d = nc.declare_dram_parameter("wq", [128, DC * DIM], BF16, isOutput=False)
    wk_d = nc.declare_dram_parameter("wk", [128, DC * DIM], BF16, isOutput=False)
    wv_d = nc.declare_dram_parameter("wv", [128, DC * DIM], BF16, isOutput=False)
    wff_d = nc.declare_dram_parameter("wff", [128, DC * DIM], BF16, isOutput=False)
    bq_d = nc.declare_dram_parameter("bq", [128, HC], F32, isOutput=False)
    bk_d = nc.declare_dram_parameter("bk", [128, HC], F32, isOutput=False)
    bffe_d = nc.declare_dram_parameter("bffe", [128, DC], F32, isOutput=False)
    # mask as 0/1: per-partition scalar [128, NKC] and 64-wide replicated bf16
    mv_d = nc.declare_dram_parameter("mv", [128, NKC], F32, isOutput=False)
    mbc_d = nc.declare_dram_parameter("mbc", [128, NKC * 64], BF16, isOutput=False)
    # out: [tt, p, oc*512] (token quarter major, partition-contiguous)
    out_d = nc.declare_dram_parameter("out", [4, 128, DC * 512], BF16, isOutput=True)

    Exp = mybir.ActivationFunctionType.Exp
    Ident = mybir.ActivationFunctionType.Identity
    ADD = mybir.AluOpType.add
    MULT = mybir.AluOpType.mult

    with TileContext(nc) as tc:
        with (
            tc.tile_pool(name="const", bufs=1) as cpool,
            tc.tile_pool(name="mm", bufs=2, space="PSUM") as mm_pool,
            tc.tile_pool(name="sps", bufs=2, space="PSUM") as s_pool,
            tc.tile_pool(name="adp", bufs=2, space="PSUM") as ad_pool,
            tc.tile_pool(name="et", bufs=7) as et_pool,
            tc.tile_pool(name="nrm", bufs=4) as nrm_pool,
            tc.tile_pool(name="ob", bufs=3) as ob_pool,
        ):
            # ---- persistent SBUF tensors ----
            # xt: token-quarter major [128, tt, dc, 512] to match the
            # pre-tiled DRAM layout (one fat descriptor per partition)
            xt_sb = cpool.tile([128, 4, DC, 512], BF16, name="xt_sb")
            wq_sb = cpool.tile([128, DC, DIM], BF16, name="wq_sb")
            wk_sb = cpool.tile([128, DC, DIM], BF16, name="wk_sb")
            wv_sb = cpool.tile([128, DC, DIM], BF16, name="wv_sb")
            wff_sb = cpool.tile([128, HC, DIM], BF16, name="wff_sb")
            qt_sb = cpool.tile([128, HC, T], BF16, name="qt_sb")
            kt_sb = cpool.tile([128, HC, T], BF16, name="kt_sb")
            v_sb = cpool.tile([128, NKC, DIM], BF16, name="v_sb")
            at_sb = cpool.tile([128, HC, T], BF16, name="at_sb")
            bq_sb = cpool.tile([128, HC], F32, name="bq_sb")
            bk_sb = cpool.tile([128, HC], F32, name="bk_sb")
            bffe_sb = cpool.tile([128, DC], F32, name="bffe_sb")
            mv_sb = cpool.tile([128, NKC], F32, name="mv_sb")
            mbc_sb = cpool.tile([128, NKC, 64], BF16, name="mbc_sb")

            # ---- PE warm-up: dummy matmuls on a memset tile run while the
            # input DMAs stream, so the HAM clock gate is already at 8/8
            # (2.4 GHz) when the first real matmul issues, and the PE is not
            # idle during the load.
            warm_sb = cpool.tile([128, 128], BF16, name="warm_sb")
            nc.vector.memset(warm_sb[:], 0.0)
            wps = mm_pool.tile([128, 512], F32, tag="mm", name="wps")
            for _ in range(26):
                nc.tensor.matmul(
                    wps[:, 0:128], warm_sb[:], warm_sb[:], start=True, stop=True
                )

            # ---- load inputs ----
            xt_v = xt_d.ap().rearrange("t p (c q) -> t p c q", q=512)
            wq_v = wq_d.ap().rearrange("p (c o) -> p c o", o=DIM)
            wk_v = wk_d.ap().rearrange("p (c o) -> p c o", o=DIM)
            wv_v = wv_d.ap().rearrange("p (c o) -> p c o", o=DIM)
            wff_v = wff_d.ap().rearrange("p (c o) -> p c o", o=DIM)
            # The first Q-projection needs wq + xt quarter 0.  Each transfer
            # covering all 128 partitions costs >=128 descriptors of ring
            # latency, so the critical first tensors are split into
            # (dc-pair x partition-half) pieces across both HWDGE queues so
            # several rings work in parallel.
            qeng = [nc.sync, nc.scalar]
            for i in range(3):
                for h in range(2):
                    p0, p1 = h * 64, (h + 1) * 64
                    qeng[h].dma_start(
                        out=wq_sb[p0:p1, 2 * i:2 * i + 2, :],
                        in_=wq_v[p0:p1, 2 * i:2 * i + 2, :],
                    )
                    qeng[1 - h].dma_start(
                        out=xt_sb[p0:p1, 0, 2 * i:2 * i + 2, :],
                        in_=xt_v[0, p0:p1, 2 * i:2 * i + 2, :],
                    )
            nc.scalar.dma_start(out=bq_sb[:], in_=bq_d.ap())
            nc.scalar.dma_start(out=bk_sb[:], in_=bk_d.ap())
            nc.scalar.dma_start(out=mv_sb[:], in_=mv_d.ap())
            for i in range(3):
                for h in range(2):
                    p0, p1 = h * 64, (h + 1) * 64
                    qeng[(i + h) % 2].dma_start(
                        out=wk_sb[p0:p1, 2 * i:2 * i + 2, :],
                        in_=wk_v[p0:p1, 2 * i:2 * i + 2, :],
                    )
            for h in range(2):
                p0, p1 = h * 64, (h + 1) * 64
                qeng[h].dma_start(out=wv_sb[p0:p1], in_=wv_v[p0:p1])
            for tt in range(1, 4):
                for h in range(2):
                    p0, p1 = h * 64, (h + 1) * 64
                    qeng[(tt + h) % 2].dma_start(
                        out=xt_sb[p0:p1, tt], in_=xt_v[tt, p0:p1]
                    )
            nc.scalar.dma_start(
                out=mbc_sb[:], in_=mbc_d.ap().rearrange("p (c o) -> p c o", o=64)
            )
            nc.scalar.dma_start(out=bffe_sb[:], in_=bffe_d.ap())
            for h in range(2):
                p0, p1 = h * 64, (h + 1) * 64
                qeng[h].dma_start(out=wff_sb[p0:p1], in_=wff_v[p0:p1])

            def proj_qk(w_sb, b_sb, o_sb, tt):
                # one token-quarter of a Q^T/K^T projection: out [hd, 512]
                for hc in range(HC):
                    ps = mm_pool.tile([128, 512], F32, tag="mm", name="ps")
                    for dc in range(DC):
                        nc.tensor.matmul(
                            ps[:],
                            w_sb[:, dc, hc * 128:(hc + 1) * 128],
                            xt_sb[:, tt, dc],
                            start=(dc == 0),
                            stop=(dc == DC - 1),
                        )
                    nc.vector.tensor_scalar(
                        out=o_sb[:, hc, tt * 512:(tt + 1) * 512],
                        in0=ps[:],
                        scalar1=b_sb[:, hc:hc + 1],
                        scalar2=None,
                        op0=ADD,
                    )

            def proj_v(kc):
                # V token-chunk [128 tokens, 768], mask folded in.
                # dc outer / half inner so consecutive matmuls share lhsT.
                tt, j0 = kc // 4, (kc % 4) * 128
                ps = [
                    mm_pool.tile([128, 384], F32, tag="mm", name="ps"),
                    mm_pool.tile([128, 384], F32, tag="mm", name="ps"),
                ]
                for dc in range(DC):
                    for half in range(2):
                        nc.tensor.matmul(
                            ps[half][:],
                            xt_sb[:, tt, dc, j0:j0 + 128],
                            wv_sb[:, dc, half * 384:(half + 1) * 384],
                            start=(dc == 0),
                            stop=(dc == DC - 1),
                        )
                for half in range(2):
                    nc.vector.tensor_scalar(
                        out=v_sb[:, kc, half * 384:(half + 1) * 384],
                        in0=ps[half][:],
                        scalar1=mv_sb[:, kc:kc + 1],
                        scalar2=None,
                        op0=MULT,
                    )

            def attn_block(blk):
                q0 = blk * 256

                # --- phase A: all scores (row-tiled 64x128, T0/T8 pack) ---
                # sp is a [128, par, kc, 256] 2-bank tile per head pair:
                # parity selects the bank (row tiles must hit different
                # banks), kc the half within it.  One EXP per head pair
                # covers all 1024 columns: fewer ACT ops, and the single
                # consumer keeps the 4 matmuls dense in the PE stream so
                # the T0/T8 pairs actually pack.
                ets = []
                for hp in range(H // 2):
                    sp = s_pool.tile([128, 2, 2, 256], F32, tag="s", name="sp")
                    for kc in range(2):
                        k0 = q0 + kc * 128
                        for par in range(2):  # alternate T0/T8 for packing
                            hr = par * 64
                            nc.tensor.matmul(
                                sp[:, par, kc],
                                kt_sb[hr:hr + 64, hp, k0:k0 + 128],
                                qt_sb[hr:hr + 64, hp, q0:q0 + 256],
                                start=True, stop=True,
                            )
                    et = et_pool.tile([128, 2, 2, 256], BF16, tag="et", name="et")
                    nc.scalar.activation(
                        et[:], sp[:], Exp, bias=0.0, scale=0.125
                    )
                    ets.append(et)
                # --- phase B: av (cols 0:256) + denominator (cols 256:512)
                # in one bank, col-tiled 128x64 T0/T1; parities alternate
                # innermost so the two col tiles stream concurrently ---
                for hp in range(H // 2):
                    ad = ad_pool.tile([128, 512], F32, tag="ad", name="ad")
                    for kc in range(2):
                        tkc = blk * 2 + kc
                        for par in range(2):
                            hr = par * 64
                            h = 2 * hp + par
                            nc.tensor.matmul(
                                ad[hr:hr + 64, 0:256],
                                v_sb[:, tkc, h * 64:(h + 1) * 64],
                                ets[hp][:, par, kc],
                                start=(kc == 0), stop=(kc == 1),
                                skip_group_check=True,
                            )
                    for kc in range(2):
                        tkc = blk * 2 + kc
                        for par in range(2):
                            hr = par * 64
                            nc.tensor.matmul(
                                ad[hr:hr + 64, 256:512],
                                mbc_sb[:, tkc],
                                ets[hp][:, par, kc],
                                start=(kc == 0), stop=(kc == 1),
                                skip_group_check=True,
                            )
                    rc = nrm_pool.tile([128, 256], F32, tag="rc", name="rc")
                    nc.vector.reciprocal_approx_fast(rc[:], ad[:, 256:512])
                    nc.vector.tensor_mul(
                        at_sb[:, hp, q0:q0 + 256], ad[:, 0:256], rc[:]
                    )

            out_v = out_d.ap().rearrange("t p (j q) -> t p j q", q=1024)

            def ff(tt):
                # oc pairs share one [128, 1024] bf16 tile; the store for a
                # pair is partition-contiguous 2KB in DRAM, split by
                # partition halves (quarters for the last unit) so several
                # rings drain it in parallel.
                ob = None
                for oc in range(DC):
                    ps = mm_pool.tile([128, 512], F32, tag="mm", name="ps")
                    for hc in range(HC):
                        nc.tensor.matmul(
                            ps[:],
                            wff_sb[:, hc, oc * 128:(oc + 1) * 128],
                            at_sb[:, hc, tt * 512:(tt + 1) * 512],
                            start=(hc == 0),
                            stop=(hc == HC - 1),
                        )
                    if oc % 2 == 0:
                        ob = ob_pool.tile([128, 1024], BF16, tag="ob", name="ob")
                    nc.scalar.activation(
                        ob[:, (oc % 2) * 512:(oc % 2) * 512 + 512], ps[:],
                        Ident, bias=bffe_sb[:, oc:oc + 1], scale=1.0
                    )
                    if oc % 2 == 1:
                        j = oc // 2
                        nsp = 4 if tt == 3 else 2
                        step = 128 // nsp
                        for h in range(nsp):
                            p0, p1 = h * step, (h + 1) * step
                            qeng[(j + h) % 2].dma_start(
                                out=out_v[tt, p0:p1, j],
                                in_=ob[p0:p1, :],
                            )

            # ---- software-pipelined emission over 4 token-quarters ----
            for u in range(4):
                proj_qk(wq_sb, bq_sb, qt_sb, u)
                proj_qk(wk_sb, bk_sb, kt_sb, u)
                for kc in range(4 * u, 4 * u + 4):
                    proj_v(kc)
                attn_block(2 * u)
                attn_block(2 * u + 1)
                ff(u)

    nc.finalize()
    return nc


def _get_nc():
    if "nc" not in _nc_cache:
        _nc_cache["nc"] = _build_nc()
    return _nc_cache["nc"]


def _tile_w(w):
    # [DIM, DIM] row-major -> [128, DC*DIM] with each partition's DC chunks
    # contiguous: element (dc*128+p, o) -> arr[p, dc*DIM+o]
    bf = ml_dtypes.bfloat16
    return np.ascontiguousarray(
        w.astype(bf).reshape(DC, 128, DIM).transpose(1, 0, 2).reshape(128, DC * DIM)
    )


def _prep_in_maps(X, mask, Wq, bq, Wk, bk, Wv, bv, Wff, bff):
    bf = ml_dtypes.bfloat16
    wq_b = _tile_w(Wq)
    wk_b = _tile_w(Wk)
    wv_b = _tile_w(Wv)
    wff_b = _tile_w(Wff)
    # per-partition bias layouts: [128, nchunks] with col = chunk
    bq_t = np.ascontiguousarray(bq.astype(np.float32).reshape(HC, 128).T)
    bk_t = np.ascontiguousarray(bk.astype(np.float32).reshape(HC, 128).T)
    bffe = (bff.astype(np.float64)
            + bv.astype(np.float64) @ Wff.astype(np.float64)).astype(np.float32)
    bffe_t = np.ascontiguousarray(bffe.reshape(DC, 128).T)

    in_maps = []
    for c in range(NCORES):
        b, s0 = divmod(c, 2)
        s0 *= T
        xt = X[b, s0:s0 + T, :].T.astype(bf)   # [DIM, T]
        # -> [tt, p, dc*512]: element (dc*128+p, tt*512+q) -> arr[tt, p, dc, q]
        xt_t = np.ascontiguousarray(
            xt.reshape(DC, 128, 4, 512).transpose(2, 1, 0, 3).reshape(4, 128, DC * 512)
        )
        mvalid = (mask[b, s0:s0 + T] > 0).astype(np.float32)  # [T] 0/1
        mv_t = np.ascontiguousarray(mvalid.reshape(NKC, 128).T)  # [128, NKC]
        mbc = np.ascontiguousarray(
            np.broadcast_to(mv_t[:, :, None], (128, NKC, 64))
            .reshape(128, NKC * 64).astype(bf))
        in_maps.append({
            "xt": xt_t, "wq": wq_b, "wk": wk_b, "wv": wv_b, "wff": wff_b,
            "bq": bq_t, "bk": bk_t, "bffe": bffe_t,
            "mv": mv_t, "mbc": mbc,
        })
    return in_maps


def _assemble(results):
    out = np.empty((B, S, DIM), np.float32)
    for c in range(NCORES):
        b, s0 = divmod(c, 2)
        s0 *= T
        # [tt, p, oc*512] -> [t, d]: out[tt*512+q, oc*128+p] = arr[tt, p, oc, q]
        a = results[c]["out"].reshape(4, 128, DC, 512)
        out[b, s0:s0 + T, :] = (
            a.transpose(0, 3, 2, 1).reshape(T, DIM).astype(np.float32)
        )
    return out


def run(trace=False, **inputs):
    nc = _get_nc()
    in_maps = _prep_in_maps(**inputs)
    res = run_bass_kernel_spmd(
        nc, in_maps, core_ids=list(range(NCORES)), trace=trace
    )
    return _assemble(res.results), res


def kernel(**inputs) -> np.ndarray:
    out, _ = run(trace=False, **inputs)
    return out

